# revision 1
# baseline (speedup 1.0000x reference)
import numpy as np

N_NODES = 50000
OUT_FEATS = 128


def kernel(h, W, b, src, dst):
    """GCN layer: relu(segment_sum((h @ W)[src], dst) + b).

    Full-input / full-output contract. Computes the dense transform once
    on the node dim, gathers per-edge messages, scatter-adds into dst
    rows, then applies bias + relu.
    """
    h = np.asarray(h, dtype=np.float32)
    W = np.asarray(W, dtype=np.float32)
    b = np.asarray(b, dtype=np.float32)
    src = np.asarray(src).astype(np.int64)
    dst = np.asarray(dst).astype(np.int64)

    hw = h @ W  # [N, out]

    agg = np.zeros((N_NODES, OUT_FEATS), dtype=np.float32)
    # sort edges by dst so the scatter-add becomes a segmented reduce
    order = np.argsort(dst, kind="stable")
    dst_s = dst[order]
    msgs = hw[src[order]]  # [E, out]
    np.add.at(agg, dst_s, msgs)

    out = agg + b[None, :]
    np.maximum(out, 0.0, out=out)
    return out



# revision 2
# speedup vs baseline: 2632.0549x; 2632.0549x over previous
"""GCN layer out = relu(segment_sum((h @ W)[src], dst) + b) on 8 trn2 NeuronCores.

Strategy (aggregate-first, no collectives):
  out = relu(segment_sum(h[src], dst) @ W + b)   (linearity lets us aggregate
  the 256-dim inputs first, then apply the dense transform per dst shard).

  - dst nodes are sharded across the 8 cores (6250 each); h is replicated.
  - each core's dst nodes are bin-packed into 49 windows x 128 psum slots,
    each window holding <= 2176 edges (17 chunks of 128); padding edges point
    at h row 0 with a -1 slot id.
  - per 128-edge chunk, h[src] rows are fetched with one indirect DMA
    (one row per SBUF partition), and the segment-sum is a one-hot matmul
    accumulated in PSUM:  psum[if_half, slot] += msgs_chunk^T @ S_chunk,
    where S_chunk[e, slot] = (dstoff[e] == iota[slot]) is built on the DVE.
  - epilogue: out^T[of, slot] = relu(W^T @ agg^T + b) on the tensor+scalar
    engines; the host inverts the slot permutation.
"""
import sys

sys.path.insert(0, "/opt/trn_rl_repo")

import numpy as np

N_NODES = 50000
N_EDGES = 800000
IN_FEATS = 256
OUT_FEATS = 128
NCORES = 8
NODES_PC = N_NODES // NCORES          # 6250
WINDOWS = 49
WIN_SLOTS = 128
CHUNKS_PW = 17
CHUNKS = WINDOWS * CHUNKS_PW          # 833
SLOTS = WINDOWS * WIN_SLOTS           # 6272
EMAX_W = CHUNKS_PW * 128              # 2176

_STATE = {}


def _host_prep(h, W, b, src, dst):
    h16 = np.ascontiguousarray(h, dtype=np.float16)
    w_sb = np.ascontiguousarray(
        np.asarray(W, dtype=np.float16).reshape(2, 128, OUT_FEATS).transpose(1, 0, 2)
    )
    bias = np.ascontiguousarray(np.asarray(b, dtype=np.float32).reshape(128, 1))
    iota = np.broadcast_to(np.arange(128, dtype=np.float16), (128, 128)).copy()

    src = np.asarray(src).astype(np.int64)
    dst = np.asarray(dst).astype(np.int64)

    order = np.argsort(dst, kind="stable")
    src_s = src[order].astype(np.int32)
    deg = np.bincount(dst, minlength=N_NODES).astype(np.int64)
    row_start = np.zeros(N_NODES + 1, dtype=np.int64)
    np.cumsum(deg, out=row_start[1:])

    idx_all = np.zeros((NCORES, 128, CHUNKS), dtype=np.int32)
    off_all = np.full((NCORES, 128, CHUNKS), -1.0, dtype=np.float16)
    dst_perm = np.full((NCORES, SLOTS), -1, dtype=np.int64)

    for c in range(NCORES):
        lo = c * NODES_PC
        d_ids = np.arange(lo, lo + NODES_PC)
        d_deg = deg[lo : lo + NODES_PC]
        rank = np.argsort(-d_deg, kind="stable")
        win_of = np.empty(NODES_PC, dtype=np.int64)
        snake = np.concatenate([np.arange(WINDOWS), np.arange(WINDOWS)[::-1]])
        win_of[rank] = snake[np.arange(NODES_PC) % (2 * WINDOWS)]
        loads = np.bincount(win_of, weights=d_deg, minlength=WINDOWS).astype(np.int64)
        counts = np.bincount(win_of, minlength=WINDOWS)
        if loads.max() > EMAX_W or counts.max() > WIN_SLOTS:
            win_of[:] = -1
            loads[:] = 0
            counts[:] = 0
            for i in rank:
                w = int(np.argmin(loads + (counts >= WIN_SLOTS) * 10**9))
                win_of[i] = w
                loads[w] += d_deg[i]
                counts[w] += 1
            assert loads.max() <= EMAX_W and counts.max() <= WIN_SLOTS

        for w in range(WINDOWS):
            members = np.where(win_of == w)[0]
            g_ids = d_ids[members]
            dst_perm[c, w * WIN_SLOTS : w * WIN_SLOTS + len(members)] = g_ids
            e_src = np.zeros(EMAX_W, dtype=np.int32)
            e_off = np.full(EMAX_W, -1.0, dtype=np.float16)
            pos = 0
            for j, g in enumerate(g_ids):
                k = int(deg[g])
                if k:
                    e_src[pos : pos + k] = src_s[row_start[g] : row_start[g] + k]
                    e_off[pos : pos + k] = j
                    pos += k
            idx_all[c, :, w * CHUNKS_PW : (w + 1) * CHUNKS_PW] = e_src.reshape(
                CHUNKS_PW, 128
            ).T
            off_all[c, :, w * CHUNKS_PW : (w + 1) * CHUNKS_PW] = e_off.reshape(
                CHUNKS_PW, 128
            ).T

    common = {"h16": h16, "w_sb": w_sb, "bias": bias, "iota": iota}
    in_maps = [
        {**common, "idx": idx_all[c], "dstoff": off_all[c]} for c in range(NCORES)
    ]
    return in_maps, dst_perm


def _build_nc():
    import concourse.bass as bass
    import concourse.bacc as bacc
    import concourse.mybir as mybir
    from concourse import tile

    nc = bacc.Bacc("TRN2", target_bir_lowering=False, debug=False, num_devices=NCORES)
    f16, f32, i32 = mybir.dt.float16, mybir.dt.float32, mybir.dt.int32

    h16 = nc.dram_tensor("h16", [N_NODES, IN_FEATS], f16, kind="ExternalInput")
    idx = nc.dram_tensor("idx", [128, CHUNKS], i32, kind="ExternalInput")
    dstoff = nc.dram_tensor("dstoff", [128, CHUNKS], f16, kind="ExternalInput")
    iota = nc.dram_tensor("iota", [128, 128], f16, kind="ExternalInput")
    w_sb = nc.dram_tensor("w_sb", [128, 2, OUT_FEATS], f16, kind="ExternalInput")
    bias = nc.dram_tensor("bias", [128, 1], f32, kind="ExternalInput")
    out = nc.dram_tensor("out", [128, SLOTS], f32, kind="ExternalOutput")

    RELU = mybir.ActivationFunctionType.Relu
    EQ = mybir.AluOpType.is_equal

    with tile.TileContext(nc) as tc:
        with (
            tc.tile_pool(name="const", bufs=1) as cpool,
            tc.tile_pool(name="msgs", bufs=3) as mpool,
            tc.tile_pool(name="sel", bufs=3) as spool,
            tc.tile_pool(name="acc", bufs=1) as apool,
            tc.tile_pool(name="ps", bufs=2, space="PSUM") as pspool,
            tc.tile_pool(name="pso", bufs=2, space="PSUM") as psopool,
        ):
            idx_t = cpool.tile([128, CHUNKS], i32, tag="idx")
            nc.sync.dma_start(out=idx_t[:], in_=idx[:])
            off_t = cpool.tile([128, CHUNKS], f16, tag="off")
            nc.sync.dma_start(out=off_t[:], in_=dstoff[:])
            iota_t = cpool.tile([128, 128], f16, tag="iota")
            nc.sync.dma_start(out=iota_t[:], in_=iota[:])
            w_t = cpool.tile([128, 2, OUT_FEATS], f16, tag="w")
            nc.sync.dma_start(out=w_t[:], in_=w_sb[:])
            b_t = cpool.tile([128, 1], f32, tag="b")
            nc.sync.dma_start(out=b_t[:], in_=bias[:])

            agg_lo = apool.tile([128, SLOTS], f16, tag="agg_lo")
            agg_hi = apool.tile([128, SLOTS], f16, tag="agg_hi")

            for w in range(WINDOWS):
                msgs = mpool.tile([128, CHUNKS_PW, IN_FEATS], f16, tag="msgs")
                sel = spool.tile([128, CHUNKS_PW, 128], f16, tag="sel")
                for cch in range(CHUNKS_PW):
                    k = w * CHUNKS_PW + cch
                    nc.gpsimd.indirect_dma_start(
                        out=msgs[:, cch, :],
                        out_offset=None,
                        in_=h16[:, :],
                        in_offset=bass.IndirectOffsetOnAxis(
                            ap=idx_t[:, k : k + 1], axis=0
                        ),
                    )
                    nc.vector.tensor_tensor(
                        out=sel[:, cch, :],
                        in0=off_t[:, k : k + 1].to_broadcast([128, 128]),
                        in1=iota_t[:, :],
                        op=EQ,
                    )
                ps_lo = pspool.tile([128, 128], f32, tag="ps_lo")
                ps_hi = pspool.tile([128, 128], f32, tag="ps_hi")
                for cch in range(CHUNKS_PW):
                    first, last = cch == 0, cch == CHUNKS_PW - 1
                    nc.tensor.matmul(
                        out=ps_lo[:],
                        lhsT=msgs[:, cch, 0:128],
                        rhs=sel[:, cch, :],
                        start=first,
                        stop=last,
                    )
                    nc.tensor.matmul(
                        out=ps_hi[:],
                        lhsT=msgs[:, cch, 128:256],
                        rhs=sel[:, cch, :],
                        start=first,
                        stop=last,
                    )
                sl = slice(w * WIN_SLOTS, (w + 1) * WIN_SLOTS)
                nc.vector.tensor_copy(out=agg_lo[:, sl], in_=ps_lo[:])
                nc.vector.tensor_copy(out=agg_hi[:, sl], in_=ps_hi[:])

            out_sb = apool.tile([128, SLOTS], f32, tag="out_sb")
            TILE_N = 512
            for st in range(0, SLOTS, TILE_N):
                wd = min(TILE_N, SLOTS - st)
                ps_o = psopool.tile([128, TILE_N], f32, tag="ps_o")
                nc.tensor.matmul(
                    out=ps_o[:, :wd],
                    lhsT=w_t[:, 0, :],
                    rhs=agg_lo[:, st : st + wd],
                    start=True,
                    stop=False,
                )
                nc.tensor.matmul(
                    out=ps_o[:, :wd],
                    lhsT=w_t[:, 1, :],
                    rhs=agg_hi[:, st : st + wd],
                    start=False,
                    stop=True,
                )
                nc.scalar.activation(
                    out=out_sb[:, st : st + wd],
                    in_=ps_o[:, :wd],
                    func=RELU,
                    bias=b_t[:, :],
                )
            nc.sync.dma_start(out=out[:], in_=out_sb[:])
    return nc


class _Runner:
    """jit once / run many wrapper around the bass2jax PJRT path."""

    def __init__(self, nc, n_cores):
        import jax
        from jax.sharding import Mesh, PartitionSpec
        from jax.experimental.shard_map import shard_map
        import concourse.mybir as mybir
        from concourse import bass2jax

        bass2jax.install_neuronx_cc_hook()
        self.jax = jax
        self.n_cores = n_cores
        partition_name = nc.partition_id_tensor.name if nc.partition_id_tensor else None
        dbg_name = nc.dbg_addr.name if nc.dbg_addr else None
        in_names, out_names, out_avals = [], [], []
        for alloc in nc.m.functions[0].allocations:
            if not isinstance(alloc, mybir.MemoryLocationSet):
                continue
            name = alloc.memorylocations[0].name
            if alloc.kind == "ExternalInput":
                if name not in (partition_name, dbg_name):
                    in_names.append(name)
            elif alloc.kind == "ExternalOutput":
                out_names.append(name)
                out_avals.append(
                    jax.core.ShapedArray(
                        tuple(alloc.tensor_shape), mybir.dt.np(alloc.dtype)
                    )
                )
        self.in_names, self.out_names, self.out_avals = in_names, out_names, out_avals
        n_params, n_outs = len(in_names), len(out_names)
        all_in_names = list(in_names) + list(out_names)
        if dbg_name is not None:
            all_in_names.append(dbg_name)
        if partition_name is not None:
            all_in_names.append(partition_name)

        def _body(*args):
            operands = list(args)
            if dbg_name is not None:
                operands.append(np.zeros((1, 2), np.uint32))
            if partition_name is not None:
                operands.append(bass2jax.partition_id_tensor())
            return tuple(
                bass2jax._bass_exec_p.bind(
                    *operands,
                    out_avals=tuple(out_avals),
                    in_names=tuple(all_in_names),
                    out_names=tuple(out_names),
                    lowering_input_output_aliases=(),
                    sim_require_finite=True,
                    sim_require_nnan=True,
                    nc=nc,
                )
            )

        devices = jax.devices()[:n_cores]
        self.mesh = Mesh(np.asarray(devices), ("core",))
        self.pspec = PartitionSpec("core")
        self._fn = jax.jit(
            shard_map(
                _body,
                mesh=self.mesh,
                in_specs=(self.pspec,) * (n_params + n_outs),
                out_specs=(self.pspec,) * n_outs,
                check_rep=False,
            ),
            keep_unused=True,
        )

    def set_inputs(self, in_maps):
        jax = self.jax
        sharding = jax.sharding.NamedSharding(self.mesh, self.pspec)
        concat = [
            np.concatenate([np.asarray(m[name]) for m in in_maps], axis=0)
            for name in self.in_names
        ]
        zeros = [
            np.zeros((self.n_cores * a.shape[0], *a.shape[1:]), a.dtype)
            for a in self.out_avals
        ]
        self._dev_inputs = [jax.device_put(x, sharding) for x in concat]
        self._zero_outs = [jax.device_put(z, sharding) for z in zeros]
        for x in self._dev_inputs + self._zero_outs:
            x.block_until_ready()

    def run(self):
        outs = self._fn(*self._dev_inputs, *self._zero_outs)
        for o in outs:
            o.block_until_ready()
        return outs

    def results(self, outs):
        per_core = []
        for c in range(self.n_cores):
            d = {}
            for i, name in enumerate(self.out_names):
                a = np.asarray(outs[i]).reshape(self.n_cores, *self.out_avals[i].shape)
                d[name] = a[c]
            per_core.append(d)
        return per_core


def _get_runner():
    if "runner" not in _STATE:
        nc = _build_nc()
        nc.compile()
        _STATE["nc"] = nc
        _STATE["runner"] = _Runner(nc, NCORES)
    return _STATE["runner"]


def simulated_exec_ns():
    """Cost-model (TimelineSim) per-core execution time estimate in ns."""
    if "sim_ns" not in _STATE:
        from concourse.timeline_sim import TimelineSim

        _get_runner()
        sim = TimelineSim(_STATE["nc"], trace=False)
        _STATE["sim_ns"] = float(sim.simulate())
    return _STATE["sim_ns"]


def kernel(h, W, b, src, dst):
    runner = _get_runner()
    in_maps, dst_perm = _host_prep(h, W, b, src, dst)
    runner.set_inputs(in_maps)
    outs = runner.run()
    results = runner.results(outs)
    out_full = np.zeros((N_NODES, OUT_FEATS), dtype=np.float32)
    for c in range(NCORES):
        o = results[c]["out"]  # [128 of, SLOTS]
        valid = dst_perm[c] >= 0
        out_full[dst_perm[c][valid]] = o.T[valid]
    return out_full
